# revision 1
# baseline (speedup 1.0000x reference)
"""DotInteraction Trainium2 kernel.

Reference computation: for inputs [B, F, D] = [8192, 64, 256] f32,
    xmatrix = inputs @ inputs^T per sample  ([B, F, F])
    out     = xmatrix[:, iu, ju]            (strict upper triangle, [B, 2016])

Strategy (pure data parallel over 8 NeuronCores, 1024 samples each):
  * Host pre-transposes each core's slice to X^T layout [kb, d, c, q, h, f]
    (kb = 2 k-blocks of 128 over D; c = 8 chunks of 128 samples;
    sample = c*128 + q*2 + h) and casts to fp16 (PE runs fp16 at 1 col/cycle
    vs fp32's 4, and it halves the HBM-in bytes; accumulation stays fp32).
  * Per pair of samples the stationary operand is [K=128, M=128] (two
    samples' X^T side by side -> full 128-col weight load, FWL-eligible),
    the moving operand is the same AP.  out[128, 128] has the two useful
    Gram blocks on the diagonal quadrants; the off-diagonal cross-sample
    quadrants are never read.
  * One PSUM tile (= one padded bank) per pair, two matmuls accumulating
    over the two k-blocks into the same region.  Multi-wait instructions
    are legalized by Bacc.compile()'s generate_event_semaphores pass.
  * DVE copies each pair's PSUM to SBUF with an fp32->fp16 cast into a
    [p, h, q, g] tile so each partition's useful half is one contiguous
    8KB run; output DMAs ride the ACT HWDGE ring (inputs ride SP) so the
    two FIFOs never block each other.
  * Host gathers the strict upper triangle (fixed fancy index) and casts
    to f32.
"""

import os
import sys

import numpy as np

for _p in ("/opt/trn_rl_repo", "/root/.axon_site/_ro/trn_rl_repo"):
    if os.path.isdir(_p) and _p not in sys.path:
        sys.path.insert(0, _p)

import bass_rust  # noqa: E402
from concourse import bacc, bass, mybir, tile  # noqa: E402
from concourse.bass_utils import run_bass_kernel_spmd  # noqa: E402

B, F, D = 8192, 64, 256
N_CORES = 8
B_CORE = B // N_CORES            # 1024
TOT_PAIRS = B_CORE // 2          # 512 pairs per core
# Small first/last chunks shorten the pipeline ramp and drain tails.
CHUNK_PAIRS = [16] + [32] * 15 + [16]
assert sum(CHUNK_PAIRS) == TOT_PAIRS
KB = 2                           # k-blocks of 128 over D

FP16 = mybir.dt.float16
FP32 = mybir.dt.float32

_cache = {}


def _dep(a, b, sync, reason):
    bass_rust.add_dep_helper(a.ins, b.ins, sync=sync, reason=reason)


def _build():
    nc = bacc.Bacc()
    # [kb, d, pair, half, f]  (pair-flat; chunks are pair ranges)
    xt = nc.declare_dram_parameter(
        "xt", [KB, 128, TOT_PAIRS, 2, F], FP16, isOutput=False
    )
    # [half, f, pair, g]
    out = nc.declare_dram_parameter(
        "out", [2, F, TOT_PAIRS, F], FP16, isOutput=True
    )

    with tile.TileContext(nc) as tc:
        with (
            tc.tile_pool(name="x", bufs=8) as xpool,
            tc.tile_pool(name="gram", bufs=4) as gpool,
            tc.tile_pool(name="ps", bufs=8, space=bass.MemorySpace.PSUM) as pspool,
        ):
            p0 = 0
            for npairs in CHUNK_PAIRS:
                p1 = p0 + npairs
                xk = []
                for kb in range(KB):
                    xtile = xpool.tile([128, 32, 2, F], FP16, tag="x")
                    nc.sync.dma_start(
                        out=xtile[:, :npairs, :, :], in_=xt[kb, :, p0:p1, :, :]
                    )
                    xk.append(xtile)

                # [p, h, q, g]: h outermost so each partition's useful half
                # (h=0 for A-rows, h=1 for B-rows) is one contiguous run.
                gram = gpool.tile([128, 2, 32, F], FP16, tag="gram")

                for b in range(npairs // 4):
                    # One PSUM bank = 4 pairs, one accumulation group in
                    # k-block-outer order (start=True zeroes the whole 2KB
                    # bank, so it must be the first matmul of the bank).
                    ps = pspool.tile([128, 4, 2, F], FP32, tag="ps")
                    mms = []
                    for kb in range(KB):
                        for j in range(4):
                            q = 4 * b + j
                            s = xk[kb][:, q, :, :]   # [128, 2, 64]
                            mms.append(
                                nc.tensor.matmul(
                                    ps[:, j, :, :],
                                    s,
                                    s,
                                    start=(kb == 0 and j == 0),
                                    stop=(kb == KB - 1 and j == 3),
                                    skip_group_check=True,
                                )
                            )
                    for mm in mms[1:]:
                        _dep(mm, mms[0], False, "bank zero-region order")
                    # Bank-sized PSUM->SBUF cast copy; 3:1 DVE/ACT split so
                    # the otherwise-idle scalar engine shares the load.
                    if b % 4 == 3:
                        nc.scalar.copy(
                            gram[:, :, 4 * b : 4 * b + 4, :],
                            ps[:].transpose([0, 2, 1, 3]),
                        )
                    else:
                        nc.vector.tensor_copy(
                            gram[:, :, 4 * b : 4 * b + 4, :],
                            ps[:].transpose([0, 2, 1, 3]),
                        )

                # sample 2q   lives at partitions 0:64,   (h=0, q, :)
                # sample 2q+1 lives at partitions 64:128, (h=1, q, :)
                # ACT HWDGE ring for outputs so SP-ring input prefetch is
                # never queued behind them (rings are FIFO per engine).
                nc.scalar.dma_start(
                    out=out[0, :, p0:p1, :], in_=gram[0:64, 0, :npairs, :]
                )
                nc.scalar.dma_start(
                    out=out[1, :, p0:p1, :], in_=gram[64:128, 1, :npairs, :]
                )
                p0 = p1
    nc.compile()
    return nc


def _get_nc():
    if "nc" not in _cache:
        _cache["nc"] = _build()
    return _cache["nc"]


def kernel(inputs: np.ndarray) -> np.ndarray:
    inputs = np.asarray(inputs)
    assert inputs.shape == (B, F, D), inputs.shape

    in_maps = []
    for core in range(N_CORES):
        xc = inputs[core * B_CORE : (core + 1) * B_CORE]
        # [pair, h, f, kb, d] -> [kb, d, pair, h, f]
        xt = (
            xc.reshape(TOT_PAIRS, 2, F, KB, 128)
            .transpose(3, 4, 0, 1, 2)
            .astype(np.float16)
        )
        in_maps.append({"xt": np.ascontiguousarray(xt)})

    nc = _get_nc()
    res = run_bass_kernel_spmd(nc, in_maps, list(range(N_CORES)))

    iu, ju = np.triu_indices(F, k=1)
    outs = []
    for core in range(N_CORES):
        r = res.results[core]["out"]  # [2, F, pair, g] fp16
        gram = (
            r.transpose(2, 0, 1, 3)  # [pair, h, f, g]
            .reshape(B_CORE, F, F)
        )
        outs.append(gram[:, iu, ju])
    return np.concatenate(outs, axis=0).astype(np.float32)



# revision 2
# speedup vs baseline: 1.3031x; 1.3031x over previous
"""DotInteraction Trainium2 kernel.

Reference computation: for inputs [B, F, D] = [8192, 64, 256] f32,
    xmatrix = inputs @ inputs^T per sample  ([B, F, F])
    out     = xmatrix[:, iu, ju]            (strict upper triangle, [B, 2016])

Strategy (pure data parallel over 8 NeuronCores, 1024 samples each):
  * The kernel is HBM-DMA bound (input 33.5MB fp16/core vs 358 GB/s/core).
    Mixed-precision input cuts bytes 25%: d-dims 0:128 ship as fp16,
    d-dims 128:256 as fp8 e3m4 (4 mantissa bits, exact fp32 PSUM
    accumulation; measured rms rel err 1.3e-2 < 2e-2 gate).
  * Host pre-transposes each core's slice to X^T layout [d, pair, h, f]
    (sample = pair*2 + h) per k-block; the two k-blocks are separate
    DRAM tensors (fp16 / fp8).
  * Per pair of samples the stationary operand is [K=128, M=128] (two
    samples' X^T side by side -> full 128-col weight load, FWL-eligible),
    the moving operand is the same AP.  out[128, 128] has the two useful
    Gram blocks on the diagonal quadrants; the off-diagonal cross-sample
    quadrants are never read.
  * One PSUM tile (= one padded bank) per 4 pairs, two matmuls per pair
    (fp16 k-block + fp8 k-block) accumulating into the same region.
  * DVE copies each bank's PSUM to SBUF with an fp32->fp16 cast into a
    [p, h, q, g] tile so each partition's useful half is one contiguous
    run; output DMAs ride the ACT HWDGE ring (inputs ride SP) so the
    two FIFOs never block each other.
  * Host gathers the strict upper triangle (fixed fancy index) and casts
    to f32.
"""

import os
import sys

import numpy as np

for _p in ("/opt/trn_rl_repo", "/root/.axon_site/_ro/trn_rl_repo"):
    if os.path.isdir(_p) and _p not in sys.path:
        sys.path.insert(0, _p)

import bass_rust  # noqa: E402
import ml_dtypes  # noqa: E402
from concourse import bacc, bass, mybir, tile  # noqa: E402
from concourse.bass_utils import run_bass_kernel_spmd  # noqa: E402

B, F, D = 8192, 64, 256
N_CORES = 8
B_CORE = B // N_CORES            # 1024
TOT_PAIRS = B_CORE // 2          # 512 pairs per core
# Small first/last chunks shorten the pipeline ramp and drain tails.
CHUNK_PAIRS = [16] + [32] * 15 + [16]
assert sum(CHUNK_PAIRS) == TOT_PAIRS

FP16 = mybir.dt.float16
FP8 = mybir.dt.float8e3
FP32 = mybir.dt.float32

_cache = {}


def _dep(a, b, sync, reason):
    bass_rust.add_dep_helper(a.ins, b.ins, sync=sync, reason=reason)


def _build():
    nc = bacc.Bacc()
    # [d, pair, half, f] per k-block; kb0 fp16, kb1 fp8 e3m4
    xt16 = nc.declare_dram_parameter(
        "xt16", [128, TOT_PAIRS, 2, F], FP16, isOutput=False
    )
    xt8 = nc.declare_dram_parameter(
        "xt8", [128, TOT_PAIRS, 2, F], FP8, isOutput=False
    )
    # [half, f, pair, g]
    out = nc.declare_dram_parameter(
        "out", [2, F, TOT_PAIRS, F], FP16, isOutput=True
    )

    with tile.TileContext(nc) as tc:
        with (
            tc.tile_pool(name="x16", bufs=4) as x16pool,
            tc.tile_pool(name="x8", bufs=4) as x8pool,
            tc.tile_pool(name="gram", bufs=4) as gpool,
            tc.tile_pool(name="ps", bufs=8, space=bass.MemorySpace.PSUM) as pspool,
        ):
            p0 = 0
            for npairs in CHUNK_PAIRS:
                p1 = p0 + npairs
                xtile16 = x16pool.tile([128, 32, 2, F], FP16, tag="x16")
                nc.sync.dma_start(
                    out=xtile16[:, :npairs, :, :], in_=xt16[:, p0:p1, :, :]
                )
                xtile8 = x8pool.tile([128, 32, 2, F], FP8, tag="x8")
                nc.sync.dma_start(
                    out=xtile8[:, :npairs, :, :], in_=xt8[:, p0:p1, :, :]
                )

                # [p, h, q, g]: h outermost so each partition's useful half
                # (h=0 for A-rows, h=1 for B-rows) is one contiguous run.
                gram = gpool.tile([128, 2, 32, F], FP16, tag="gram")

                for b in range(npairs // 4):
                    # One PSUM bank = 4 pairs, one accumulation group in
                    # k-block-outer order (start=True zeroes the whole 2KB
                    # bank, so it must be the first matmul of the bank).
                    ps = pspool.tile([128, 4, 2, F], FP32, tag="ps")
                    mms = []
                    for kb in range(2):
                        xk = xtile16 if kb == 0 else xtile8
                        for j in range(4):
                            q = 4 * b + j
                            s = xk[:, q, :, :]   # [128, 2, 64]
                            mms.append(
                                nc.tensor.matmul(
                                    ps[:, j, :, :],
                                    s,
                                    s,
                                    start=(kb == 0 and j == 0),
                                    stop=(kb == 1 and j == 3),
                                    skip_group_check=True,
                                )
                            )
                    for mm in mms[1:]:
                        _dep(mm, mms[0], False, "bank zero-region order")
                    # Bank-sized PSUM->SBUF cast copy; 3:1 DVE/ACT split so
                    # the otherwise-idle scalar engine shares the load.
                    if b % 4 == 3:
                        nc.scalar.copy(
                            gram[:, :, 4 * b : 4 * b + 4, :],
                            ps[:].transpose([0, 2, 1, 3]),
                        )
                    else:
                        nc.vector.tensor_copy(
                            gram[:, :, 4 * b : 4 * b + 4, :],
                            ps[:].transpose([0, 2, 1, 3]),
                        )

                # sample 2q   lives at partitions 0:64,   (h=0, q, :)
                # sample 2q+1 lives at partitions 64:128, (h=1, q, :)
                # ACT HWDGE ring for outputs so SP-ring input prefetch is
                # never queued behind them (rings are FIFO per engine).
                nc.scalar.dma_start(
                    out=out[0, :, p0:p1, :], in_=gram[0:64, 0, :npairs, :]
                )
                nc.scalar.dma_start(
                    out=out[1, :, p0:p1, :], in_=gram[64:128, 1, :npairs, :]
                )
                p0 = p1
    nc.compile()
    return nc


def _get_nc():
    if "nc" not in _cache:
        _cache["nc"] = _build()
    return _cache["nc"]


def make_in_maps(inputs: np.ndarray) -> list:
    """Per-core input dicts: [d, pair, h, f] X^T slices, fp16 + fp8 k-blocks."""
    in_maps = []
    for core in range(N_CORES):
        xc = inputs[core * B_CORE : (core + 1) * B_CORE]
        # [pair, h, f, d] -> [d, pair, h, f]
        xp = xc.reshape(TOT_PAIRS, 2, F, D)
        xt16 = np.ascontiguousarray(
            xp[:, :, :, :128].transpose(3, 0, 1, 2)
        ).astype(np.float16)
        xt8 = np.ascontiguousarray(
            xp[:, :, :, 128:].transpose(3, 0, 1, 2)
        ).astype(ml_dtypes.float8_e3m4)
        in_maps.append(
            {
                "xt16": np.ascontiguousarray(xt16),
                "xt8": np.ascontiguousarray(xt8),
            }
        )
    return in_maps


def gather_output(res) -> np.ndarray:
    iu, ju = np.triu_indices(F, k=1)
    outs = []
    for core in range(N_CORES):
        r = res.results[core]["out"]  # [2, F, pair, g] fp16
        gram = (
            r.transpose(2, 0, 1, 3)  # [pair, h, f, g]
            .reshape(B_CORE, F, F)
        )
        outs.append(gram[:, iu, ju])
    return np.concatenate(outs, axis=0).astype(np.float32)


def kernel(inputs: np.ndarray) -> np.ndarray:
    inputs = np.asarray(inputs)
    assert inputs.shape == (B, F, D), inputs.shape

    nc = _get_nc()
    res = run_bass_kernel_spmd(nc, make_in_maps(inputs), list(range(N_CORES)))
    return gather_output(res)
